# revision 55
# baseline (speedup 1.0000x reference)
"""Trainium2 Bass kernel for nn_MultiHeadSelfAttention (N=2, S=2048, E=1024, H=16).

Sharding: heads+batch tensor-parallel over 8 cores. Core c handles batch
n = c // 4 and 4 heads h in [4*(c%4), 4*(c%4)+4), processed as 2 pairs.
fc_out is row-parallel: each core computes a partial y over its 256
embedding dims, and the host sums 4 partials per batch and adds bias.

PE row-cycle minimization:
  - energy (contraction D=64, half the array rows) is ROW-TILED: the two
    heads of a pair run concurrently in row groups 0-63 / 64-127 -> 2x.
    On top of that, q ships as UNSCALED fp8e4 (the moving operand streams
    2 elem/cycle on TRN2 -> energy matmul instructions halve again), while
    k stays bf16 (stationary, exact) and carries the S_Q prescale so the
    energies still arrive pre-scaled for the exp paths. Measured: mixed
    bf16-stationary x fp8-moving matmuls are numerically exact f32-accum
    and run at the fp8 stream rate (~92.7ns vs ~185ns per row-tiled half).
    One-sided q quantization costs ~1.16e-2 rel err (gate 2e-2).
  - attV uses a [V | ones x8] column trick: the denominator falls out
    REPLICATED on rows 64-71 of the output (extra out partitions are free
    - matmul cost is free-dim-bound), so the norm needs only a single
    8->64 broadcast DMA instead of a 2-hop 1->8->64 chain. attV must
    stay bf16: fp8 'a' (3-bit mantissa) pushes rel err past the gate
    (simulated 2.4-3.2e-2), and DoubleRow fp8 requires both operands fp8.
  - fc stacks the pair's 128 embedding dims as a single K=128 contraction.
    Matmul out free dim is capped at one PSUM bank (512 f32) by the ISA,
    so no wider instruction merging is possible anywhere.

Per-pair pipeline:
  e_h[k, q]   = (S_Q*KT_h).T @ Q8T_h  row-tiled pair, PSUM [128, 512]
  a_h = exp(e_h * 32/32)              ScalarE / custom-DVE, bf16 out
  o_h/den     = [V_h | 1x8].T @ a_h   accumulate 16 key chunks
  X^T_h = o_h * (1/den)               single 8->64 DMA bcast + DVE recip/
                                      mul, drained one op per group (no
                                      bursts; npool ring 6 so 3 norm
                                      instances overlap their drains)
  y += Xpair^T.T @ Wpair^T            K=128 per pair, 2 pairs in PSUM;
                                      yp stores alternate gpsimd/sync DMA
                                      queues (the 4MB/rep store was
                                      serializing on one queue ~12.6us,
                                      co-binding the fc phase)

Scheduling (hw-swept): attV defers TWO groups behind its exp (absorbs exp
jitter before the PE needs the a-tile); v tiles are SBUF-resident across
reps (no per-rep DMA); e-PSUM ring stays at 3x[128,1024] — dropping to 2
costs +55us (the energy->exp->e-ring recycle is the pacing loop of the
attention phase). Norm osb copies alternate ScalarE/DVE (norm_copy=alt)
and the y staging ring is 6 deep — both A/B-swept wins at the final
balance. DVE has no divide ALU op, so the norm stays recip+mul. The DVE
slot pattern is placement-critical: rotating it by one costs +30us.
"""

import time

import numpy as np

N_CORES = 8
NB = 2          # batch
S = 2048        # sequence length
E = 1024        # embed size
H = 16          # heads
D = 64          # head dim
HPC = 4         # heads per core
NPAIR = 2       # head pairs per core
SCALE = float(1.0 / np.sqrt(E))  # softmax scale (embed_size**0.5)

KC = S // 128   # 16 contraction chunks of 128 keys
QB = S // 512   # 4 query blocks of 512

# exp(x) ~= p(t)^8, p monic cubic, t = x pre-scaled by S_Q on the host.
# ScalarE chunks use exp(t * ACT_SCALE) (exact); VectorE chunks use the
# 8-stage custom DVE polynomial (rel err ~2.6e-4).
ALPHA = 6.0 ** (1.0 / 3.0)
S_Q = float(SCALE / (8.0 * ALPHA))   # host pre-scale on Q (t = S_Q/SCALE * x)
ACT_SCALE = float(8.0 * ALPHA)
B2, B1, B0 = 1.6574587989430332, 1.8171403999384372, 0.9999891634709047


def register_exp_op():
    """Register the EXP_POLY8_ANT custom DVE op (idempotent)."""
    import concourse.dve_ops as dve_ops
    from concourse.dve_ops import OPS, DveOp
    from concourse.dve_spec import C0, C1, C2, Spec, Src0, _has_src1, lower, sq
    from concourse.dve_uop import DveOpSpec

    name = "EXP_POLY8_ANT"
    for op in OPS:
        if op.name == name:
            return op

    _p = ((Src0 + C0) * Src0 + C1) * Src0 + C2
    _body = sq(sq(sq(_p)))

    def _ref(in0, in1, s0, s1, imm2):
        p = ((in0 + s0) * in0 + s1) * in0 + imm2
        return ((p ** 2) ** 2) ** 2

    spec = Spec(body=_body, reference=_ref)
    opcode = dve_ops._CUSTOM_DVE_ROW_BASE + len(OPS)
    shas = {}
    for ver in ("v3", "v4"):
        d = DveOpSpec(
            name=name, opcode=opcode, uops=lower(spec, ver=ver),
            rd1_en=_has_src1(spec),
        )
        shas[ver] = d.sha(ver)
    op = DveOp(name, spec, subdim=False, uops_sha=shas)
    OPS.append(op)
    dve_ops._SUB_OPCODE_FOR_NAME[name] = opcode
    dve_ops.CUSTOM_DVE_SPECS[name] = spec
    return op


DVE_SLOTS = frozenset({0, 2, 4, 5, 7, 9, 11, 12, 14})  # 9/16 of exps on DVE: ScalarE became the tighter engine once fp8-q halved the energy matmuls (A/B-swept on hw)


def build_kernel(reps=1, dve_slots=DVE_SLOTS, abufs=10, ybufs=6, abl=frozenset(),
                 norm_copy="alt", ebufs=3, att_defer=2, norm_div=False,
                 v_resident=True, nbufs=6, yq="half"):
    """abl: timing-ablation flags (numerics become garbage, graph stays valid):
    'attv' = only 2/16 attV chunks; 'exp' = skip exps (attV reads static a);
    'energy' = skip energy matmuls (exps read static sbuf); 'fc' = 2/4 fc
    matmuls; 'norm' = skip norm (fc reads pre-zeroed xt)."""
    import contextlib

    import concourse.bacc as bacc
    import concourse.bass as bass
    import concourse.tile as tile
    from concourse import mybir
    from concourse.alu_op_type import AluOpType

    F32 = mybir.dt.float32
    BF16 = mybir.dt.bfloat16
    FP8 = mybir.dt.float8e4

    exp_op = register_exp_op()

    nc = bacc.Bacc("TRN2", target_bir_lowering=False, num_devices=N_CORES)

    # pair-stacked layouts: rows 0-63 = head 2p, rows 64-127 = head 2p+1
    # q is fp8 (moving operand streams 2 elem/cycle -> energy matmuls 2x);
    # the S_Q prescale lives on k (bf16, exact same rounding count as before).
    qt = nc.dram_tensor("qt", [NPAIR, 128, S], FP8, kind="ExternalInput")
    kt = nc.dram_tensor("kt", [NPAIR, 128, S], BF16, kind="ExternalInput")
    # v carries EIGHT ones columns: the attV matmul then writes den to psum
    # rows 64-71 (free: matmul cost is free-dim-bound, extra out partitions
    # are free), eliminating the 1->8 broadcast DMA hop from the norm chain
    vb = nc.dram_tensor("vb", [HPC, S, D + 8], BF16, kind="ExternalInput")
    wt = nc.dram_tensor("wt", [NPAIR, 128, E], BF16, kind="ExternalInput")
    tok = nc.dram_tensor("tok", [1, 128], F32, kind="ExternalInput")
    yp = nc.dram_tensor("yp", [S, E], BF16, kind="ExternalOutput")
    tok_out = nc.dram_tensor("tok_out", [1, 128], F32, kind="ExternalOutput")

    with tile.TileContext(nc) as tc:
        with contextlib.ExitStack() as ctx:
            singles = ctx.enter_context(tc.tile_pool(name="singles", bufs=1))
            vpool = ctx.enter_context(tc.tile_pool(name="vpool", bufs=2))
            epool = ctx.enter_context(
                tc.tile_pool(name="epool", bufs=ebufs, space="PSUM")
            )
            opool = ctx.enter_context(
                tc.tile_pool(name="opool", bufs=2, space="PSUM")
            )
            apool = ctx.enter_context(tc.tile_pool(name="apool", bufs=abufs))
            npool = ctx.enter_context(tc.tile_pool(name="npool", bufs=nbufs))
            ysb_pool = ctx.enter_context(tc.tile_pool(name="ysb", bufs=ybufs))

            # token passthrough for timing chains
            tok_sb = singles.tile([1, 128], F32)
            nc.gpsimd.dma_start(out=tok_sb, in_=tok[:, :])
            nc.gpsimd.dma_start(out=tok_out[:, :], in_=tok_sb)
            # dummy exp: hoists the ACT_TABLE_LOAD out of the reps loop
            warm_exp = singles.tile([1, 128], BF16)
            nc.scalar.activation(
                out=warm_exp, in_=tok_sb,
                func=mybir.ActivationFunctionType.Exp, scale=1.0,
            )

            # resident inputs (pair-stacked on partitions)
            qt_sb, kt_sb, wt_sb, xt_sb = [], [], [], []
            for p in range(NPAIR):
                q_t = singles.tile([128, S], FP8, tag=f"qt{p}")
                nc.sync.dma_start(out=q_t, in_=qt[p])
                qt_sb.append(q_t)
                k_t = singles.tile([128, S], BF16, tag=f"kt{p}")
                nc.sync.dma_start(out=k_t, in_=kt[p])
                kt_sb.append(k_t)
                w_t = singles.tile([128, E], BF16, tag=f"wt{p}")
                nc.sync.dma_start(out=w_t, in_=wt[p])
                wt_sb.append(w_t)
                # per-(pair, qb) x tiles: fc readers then only wait on the
                # norm that wrote their own query block, not all 8 norms
                xq = []
                for qb in range(QB):
                    x_t = singles.tile([128, 512], BF16, tag=f"xt{p}_{qb}")
                    if "norm" in abl:
                        nc.vector.memset(x_t, 0.0)
                    xq.append(x_t)
                xt_sb.append(xq)

            vres = []
            if v_resident:
                for h in range(HPC):
                    v_r = singles.tile([128, KC, D + 8], BF16, tag=f"vres{h}")
                    nc.sync.dma_start(
                        out=v_r,
                        in_=vb[h].rearrange("(kc q) c -> q kc c", q=128),
                    )
                    vres.append(v_r)

            if "exp" in abl:
                abl_a0 = singles.tile([128, 1024], BF16, tag="abl_a0")
                nc.vector.memset(abl_a0, 0.25)
                abl_a1 = singles.tile([128, 1024], BF16, tag="abl_a1")
                nc.vector.memset(abl_a1, 0.25)
            if "energy" in abl:
                abl_e = singles.tile([128, 1024], F32, tag="abl_e")
                nc.vector.memset(abl_e, 0.125)

            unroll = 4 if reps > 1 and reps % 4 == 0 else (2 if reps > 1 and reps % 2 == 0 else 1)
            loop_cm = (
                tc.For_i(0, reps // unroll, 1)
                if reps > unroll
                else contextlib.nullcontext()
            )
            ctx.enter_context(loop_cm)

            from collections import deque

            for _u in range(unroll):
                exp_idx = 0
                att_q = deque()   # deferred attV groups (att_defer behind)
                norm_work = deque()  # one norm op drained per group (burst smoothing)

                def make_att(g, v0_t, v1_t, a0_t, a1_t, o0_t, o1_t):
                    def emit():
                        for j in range(2):
                            kc = 2 * g + j
                            if "attv" in abl and kc not in (0, KC - 1):
                                continue
                            nc.tensor.matmul(
                                o0_t,
                                lhsT=(v0_t[:, kc, :]),
                                rhs=(a0_t[:, j * 512 : (j + 1) * 512]),
                                start=(kc == 0), stop=(kc == KC - 1),
                            )
                            nc.tensor.matmul(
                                o1_t,
                                lhsT=(v1_t[:, kc, :]),
                                rhs=(a1_t[:, j * 512 : (j + 1) * 512]),
                                start=(kc == 0), stop=(kc == KC - 1),
                            )
                    return emit

                def make_norm_thunks(p, qb, o0_t, o1_t):
                    # 6 thunks drained one per group: copy+bcast per o tile
                    # (freeing the o ring fast), then recips, then muls.
                    # Smoothing avoids the 2.4us DVE burst that stalled the
                    # exp pipeline (and with it attV / the PE stream).
                    otiles = (o0_t, o1_t)
                    osbs = [None, None]
                    bcasts = [None, None]
                    recs = [None, None]

                    def cp(i):
                        def t():
                            osb = npool.tile([D + 8, 512], F32, tag="osb")
                            if norm_copy == "scalar" or (
                                norm_copy == "alt" and i == 0
                            ):
                                nc.scalar.copy(out=osb, in_=otiles[i])
                            else:
                                nc.vector.tensor_copy(osb, otiles[i])
                            osbs[i] = osb
                            # den already sits on 8 partitions (rows 64-71,
                            # written by the 8 ones columns of v): a single
                            # 8 -> 64 DMA with 8 parallel source partitions
                            # finishes the broadcast.
                            bc8 = osb[D : D + 8, :]
                            bcast = npool.tile([D, 512], F32, tag="bcast")
                            bc8_rep = bass.AP(
                                tensor=bc8.tensor,
                                offset=bc8.offset,
                                ap=[list(bc8.ap[0]), [0, 8]]
                                + [list(x) for x in bc8.ap[1:]],
                            )
                            nc.sync.dma_start(out=bcast, in_=bc8_rep)
                            bcasts[i] = bcast
                        return t

                    def rc(i):
                        def t():
                            # Deprioritized: the recip waits on the broadcast
                            # DMAs; exps queued behind it in the DVE FIFO
                            # would head-of-line block and stall attV.
                            with tc.high_priority(offset=-64):
                                rec = npool.tile([D, 512], F32, tag="rec")
                                nc.vector.reciprocal_approx_fast(
                                    out=rec, in_=bcasts[i]
                                )
                                recs[i] = rec
                        return t

                    def ml(i):
                        def t():
                            with tc.high_priority(offset=-64):
                                nc.vector.tensor_mul(
                                    out=xt_sb[p][qb][i * D : (i + 1) * D, :],
                                    in0=osbs[i][0:D, :],
                                    in1=recs[i],
                                )
                        return t

                    def dv(i):
                        def t():
                            # fused o/den in one DVE tensor_tensor divide
                            with tc.high_priority(offset=-64):
                                nc.vector.tensor_tensor(
                                    out=xt_sb[p][qb][i * D : (i + 1) * D, :],
                                    in0=osbs[i][0:D, :],
                                    in1=bcasts[i],
                                    op=AluOpType.divide,
                                )
                        return t

                    if norm_div:
                        return [cp(0), cp(1), dv(0), dv(1)]
                    return [cp(0), cp(1), rc(0), rc(1), ml(0), ml(1)]

                for p in range(NPAIR):
                    if v_resident:
                        # all 4 heads' v tiles fit SBUF (8.3KB/partition):
                        # load once, no per-rep DMA or WAR coupling
                        v0_t, v1_t = vres[2 * p], vres[2 * p + 1]
                    else:
                        # V tiles for this pair's heads: [128, kc, 65] each
                        v0_t = vpool.tile([128, KC, D + 8], BF16, tag="v0")
                        nc.sync.dma_start(
                            out=v0_t,
                            in_=vb[2 * p].rearrange("(kc q) c -> q kc c", q=128),
                        )
                        v1_t = vpool.tile([128, KC, D + 8], BF16, tag="v1")
                        nc.sync.dma_start(
                            out=v1_t,
                            in_=vb[2 * p + 1].rearrange("(kc q) c -> q kc c", q=128),
                        )
                    for qb in range(QB):
                        o0_t = opool.tile([D + 8, 512], F32, tag="o", name="o0")
                        o1_t = opool.tile([D + 8, 512], F32, tag="o", name="o1")
                        # interleaved: per group g (2 key chunks), emit the
                        # row-tiled energy pair matmuls + exps for g, then the
                        # attV matmuls for g-1 (one group behind, so the exps
                        # have a group of slack). Keeps all 3 engines fed.
                        for g in range(KC // 2):
                            if "energy" not in abl:
                                e0_t = epool.tile([128, 1024], F32, tag="e", name="e0")
                                e1_t = epool.tile([128, 1024], F32, tag="e", name="e1")
                                for j in range(2):
                                    kc = 2 * g + j
                                    # row-tiled energy: both heads of the pair run
                                    # concurrently in row groups 0-63 / 64-127
                                    nc.tensor.matmul(
                                        e0_t[:, j * 512 : (j + 1) * 512],
                                        lhsT=(kt_sb[p][0:D, kc * 128 : (kc + 1) * 128]),
                                        rhs=(qt_sb[p][0:D, qb * 512 : (qb + 1) * 512]),
                                        start=True, stop=True,
                                        tile_position=(0, 0),
                                    )
                                    nc.tensor.matmul(
                                        e1_t[:, j * 512 : (j + 1) * 512],
                                        lhsT=(
                                            kt_sb[p][D : 2 * D, kc * 128 : (kc + 1) * 128]
                                        ),
                                        rhs=(qt_sb[p][D : 2 * D, qb * 512 : (qb + 1) * 512]),
                                        start=True, stop=True,
                                        tile_position=(64, 0),
                                    )
                            else:
                                e0_t = e1_t = abl_e
                            if "exp" not in abl:
                                a0_t = apool.tile([128, 1024], BF16, tag="a0")
                                a1_t = apool.tile([128, 1024], BF16, tag="a1")
                                for e_t, a_t in ((e0_t, a0_t), (e1_t, a1_t)):
                                    if (exp_idx % 16) not in dve_slots:
                                        nc.scalar.activation(
                                            out=a_t,
                                            in_=e_t,
                                            func=mybir.ActivationFunctionType.Exp,
                                            scale=ACT_SCALE,
                                        )
                                    else:
                                        nc.vector._custom_dve(
                                            exp_op, out=a_t, in0=e_t,
                                            s0=B2, s1=B1, imm2=B0,
                                        )
                                    exp_idx += 1
                            else:
                                a0_t, a1_t = abl_a0, abl_a1
                            if len(att_q) >= att_defer:
                                att_q.popleft()()
                            att_q.append(make_att(g, v0_t, v1_t, a0_t, a1_t,
                                                  o0_t, o1_t))
                            # drain norm work AFTER the deferred attV flush
                            # (copies must follow the block's final attV
                            # chunks, which flush at g = att_defer-1). Both
                            # copies go at the first drain group — before the
                            # next attV reuses the o ring — then the DVE ops
                            # smooth out one per group to avoid bursts.
                            if norm_work and g >= att_defer - 1:
                                norm_work.popleft()()
                                if g == att_defer - 1 and norm_work:
                                    norm_work.popleft()()
                        if "norm" not in abl:
                            norm_work.extend(make_norm_thunks(p, qb, o0_t, o1_t))
                # flush the tail: last attV groups, then the remaining norms
                while att_q:
                    att_q.popleft()()
                while norm_work:
                    norm_work.popleft()()

                # fc phase: y[q, f] partial over this core's 2 head-pairs.
                # y PSUM tiles reuse the energy tag (epool) to stay in 8 banks;
                # bf16 stores go out on the (otherwise idle) gpsimd DMA queue so
                # they don't serialize ahead of the next rep's V loads.
                fc_pairs = 1 if "fc" in abl else NPAIR
                for q128 in range(S // 128):
                    y_t = epool.tile([128, 1024], F32, tag="e", name="y_t")
                    for f in range(E // 512):
                        for p in range(fc_pairs):
                            nc.tensor.matmul(
                                y_t[:, f * 512 : (f + 1) * 512],
                                lhsT=(
                                    xt_sb[p][q128 // 4][
                                        :, (q128 % 4) * 128 : (q128 % 4 + 1) * 128
                                    ]
                                ),
                                rhs=(wt_sb[p][:, f * 512 : (f + 1) * 512]),
                                start=(p == 0),
                                stop=(p == fc_pairs - 1),
                            )
                    y_sb = ysb_pool.tile([128, 1024], BF16)
                    if q128 % 2 == 0:
                        nc.scalar.copy(out=y_sb, in_=y_t)
                    else:
                        nc.vector.tensor_copy(y_sb, y_t)
                    # split output DMA across two queues: sync takes the
                    # FIRST half (drains early, so it is empty again when the
                    # next rep's norm bcasts arrive), gpsimd takes the second
                    # half (nothing else uses it at the rep boundary)
                    if yq == "half":
                        dma_q = nc.sync if q128 < 8 else nc.gpsimd
                    else:
                        dma_q = nc.gpsimd if q128 % 2 == 0 else nc.sync
                    dma_q.dma_start(
                        out=yp[q128 * 128 : (q128 + 1) * 128, :],
                        in_=y_sb,
                    )
    nc.compile()
    return nc


class SpmdRunner:
    """Build one jitted shard_map callable over 8 cores; reusable for timing."""

    def __init__(self, nc, n_cores):
        import jax
        from jax.experimental.shard_map import shard_map
        from jax.sharding import Mesh, PartitionSpec

        from concourse import mybir
        from concourse.bass2jax import _bass_exec_p, install_neuronx_cc_hook
        from concourse.bass2jax import partition_id_tensor as _pid

        install_neuronx_cc_hook()
        self.jax = jax
        self.nc = nc
        self.n_cores = n_cores
        self.PartitionSpec = PartitionSpec

        partition_name = nc.partition_id_tensor.name if nc.partition_id_tensor else None
        in_names, out_names, out_avals = [], [], []
        for alloc in nc.m.functions[0].allocations:
            if not isinstance(alloc, mybir.MemoryLocationSet):
                continue
            name = alloc.memorylocations[0].name
            if alloc.kind == "ExternalInput":
                if name != partition_name:
                    in_names.append(name)
            elif alloc.kind == "ExternalOutput":
                out_names.append(name)
                shape = tuple(alloc.tensor_shape)
                dtype = mybir.dt.np(alloc.dtype)
                out_avals.append(jax.core.ShapedArray(shape, dtype))
        self.in_names = in_names
        self.out_names = out_names
        self.out_avals = out_avals
        n_params = len(in_names)
        n_outs = len(out_avals)

        all_in_names = list(in_names) + list(out_names)
        if partition_name is not None:
            all_in_names.append(partition_name)

        def _body(*args):
            operands = list(args)
            if partition_name is not None:
                operands.append(_pid())
            outs = _bass_exec_p.bind(
                *operands,
                out_avals=tuple(out_avals),
                in_names=tuple(all_in_names),
                out_names=tuple(out_names),
                lowering_input_output_aliases=(),
                sim_require_finite=True,
                sim_require_nnan=True,
                nc=nc,
            )
            return tuple(outs)

        self._body = _body
        devices = jax.devices()[:n_cores]
        assert len(devices) == n_cores
        self.mesh = Mesh(np.asarray(devices), ("core",))
        in_specs = (PartitionSpec("core"),) * (n_params + n_outs)
        out_specs = (PartitionSpec("core"),) * n_outs
        self.fn = jax.jit(
            shard_map(
                _body,
                mesh=self.mesh,
                in_specs=in_specs,
                out_specs=out_specs,
                check_rep=False,
            ),
            keep_unused=True,
        )
        self._chain_fns = {}

    def prepare(self, in_maps):
        jax = self.jax
        n = self.n_cores
        concat_in = [
            np.concatenate([np.asarray(in_maps[c][name]) for c in range(n)], axis=0)
            for name in self.in_names
        ]
        concat_zeros = [
            np.zeros((n * a.shape[0], *a.shape[1:]), a.dtype) for a in self.out_avals
        ]
        sharding = jax.sharding.NamedSharding(self.mesh, self.PartitionSpec("core"))
        self.dev_args = [jax.device_put(a, sharding) for a in concat_in + concat_zeros]
        return self.dev_args

    def run(self):
        outs = self.fn(*self.dev_args)
        self.jax.block_until_ready(outs)
        return outs

    def results(self, outs):
        n = self.n_cores
        res = []
        for c in range(n):
            d = {}
            for i, name in enumerate(self.out_names):
                a = np.asarray(outs[i])
                d[name] = a.reshape(n, *self.out_avals[i].shape)[c]
            res.append(d)
        return res

    # ---- timing support: chain K invocations through the tok tensor ----
    def chain_fn(self, k):
        if k in self._chain_fns:
            return self._chain_fns[k]
        jax = self.jax
        from jax.experimental.shard_map import shard_map

        tok_in_idx = self.in_names.index("tok")
        tok_out_idx = self.out_names.index("tok_out")
        n_params = len(self.in_names)

        def _chained(*args):
            args = list(args)
            outs = None
            for _ in range(k):
                outs = self._body(*args)
                args[tok_in_idx] = outs[tok_out_idx]
            return tuple(outs)

        in_specs = (self.PartitionSpec("core"),) * (n_params + len(self.out_names))
        out_specs = (self.PartitionSpec("core"),) * len(self.out_names)
        fn = jax.jit(
            shard_map(
                _chained,
                mesh=self.mesh,
                in_specs=in_specs,
                out_specs=out_specs,
                check_rep=False,
            ),
            keep_unused=True,
        )
        self._chain_fns[k] = fn
        return fn

    def time_chain(self, k, iters=8, warmup=2):
        fn = self.chain_fn(k)
        for _ in range(warmup):
            self.jax.block_until_ready(fn(*self.dev_args))
        ts = []
        for _ in range(iters):
            t0 = time.perf_counter()
            self.jax.block_until_ready(fn(*self.dev_args))
            ts.append(time.perf_counter() - t0)
        return min(ts)


def shard_inputs(values, keys, query, W_out):
    """Build the 8 per-core input maps (host-side layout prep)."""
    import ml_dtypes

    BF = ml_dtypes.bfloat16
    F8 = ml_dtypes.float8_e4m3
    v4 = np.asarray(values, np.float32).reshape(NB, S, H, D)
    k4 = np.asarray(keys, np.float32).reshape(NB, S, H, D)
    q4 = np.asarray(query, np.float32).reshape(NB, S, H, D)
    W_out = np.asarray(W_out, np.float32)
    in_maps = []
    tok = np.zeros((1, 128), np.float32)
    for c in range(N_CORES):
        n = c // 4
        h0 = HPC * (c % 4)
        # [HPC, D, S] per-head transposed views. q ships unscaled in fp8
        # (moving operand); k carries the S_Q prescale in bf16 so energies
        # still arrive as t for the exp paths.
        qh = np.ascontiguousarray(q4[n, :, h0 : h0 + HPC, :].transpose(1, 2, 0))
        kh = np.ascontiguousarray(
            k4[n, :, h0 : h0 + HPC, :].transpose(1, 2, 0) * np.float32(S_Q)
        )
        # pair-stack: [NPAIR, 2*D, S]
        qt = qh.reshape(NPAIR, 2 * D, S).astype(F8)
        kt = kh.reshape(NPAIR, 2 * D, S).astype(BF)
        vb = np.concatenate(
            [
                np.ascontiguousarray(v4[n, :, h0 : h0 + HPC, :].transpose(1, 0, 2)),
                np.ones((HPC, S, 8), np.float32),
            ],
            axis=2,
        ).astype(BF)  # [HPC, S, D+8]: 8 ones cols -> den lands on 8 psum rows
        wt = np.ascontiguousarray(
            W_out[:, (h0 * D) : (h0 + HPC) * D].T.reshape(NPAIR, 2 * D, E)
        ).astype(BF)
        in_maps.append({"qt": qt, "kt": kt, "vb": vb, "wt": wt, "tok": tok})
    return in_maps


_CACHE = {}


def get_runner():
    if "runner" not in _CACHE:
        nc = build_kernel()
        _CACHE["runner"] = SpmdRunner(nc, N_CORES)
    return _CACHE["runner"]


def kernel(values, keys, query, W_out, b_out):
    runner = get_runner()
    in_maps = shard_inputs(values, keys, query, W_out)
    runner.prepare(in_maps)
    outs = runner.run()
    res = runner.results(outs)
    y = np.zeros((NB, S, E), np.float32)
    for c in range(N_CORES):
        y[c // 4] += np.asarray(res[c]["yp"], dtype=np.float32)
    y += np.asarray(b_out, np.float32)[None, None, :]
    return y



# revision 57
# speedup vs baseline: 1.0671x; 1.0671x over previous
"""Trainium2 Bass kernel for nn_MultiHeadSelfAttention (N=2, S=2048, E=1024, H=16).

Sharding: heads+batch tensor-parallel over 8 cores. Core c handles batch
n = c // 4 and 4 heads h in [4*(c%4), 4*(c%4)+4), processed as 2 pairs.
fc_out is row-parallel: each core computes a partial y over its 256
embedding dims, and the host sums 4 partials per batch and adds bias.

PE row-cycle minimization:
  - energy (contraction D=64, half the array rows) is ROW-TILED: the two
    heads of a pair run concurrently in row groups 0-63 / 64-127 -> 2x.
    On top of that, q ships as UNSCALED fp8e4 (the moving operand streams
    2 elem/cycle on TRN2 -> energy matmul instructions halve again), while
    k stays bf16 (stationary, exact) and carries the S_Q prescale so the
    energies still arrive pre-scaled for the exp paths. Measured: mixed
    bf16-stationary x fp8-moving matmuls are numerically exact f32-accum
    and run at the fp8 stream rate (~92.7ns vs ~185ns per row-tiled half).
    One-sided q quantization costs ~1.16e-2 rel err (gate 2e-2).
  - attV uses a [V | ones x8] column trick: the denominator falls out
    REPLICATED on rows 64-71 of the output (extra out partitions are free
    - matmul cost is free-dim-bound), so the norm needs only a single
    8->64 broadcast DMA instead of a 2-hop 1->8->64 chain. attV must
    stay bf16: fp8 'a' (3-bit mantissa) pushes rel err past the gate
    (simulated 2.4-3.2e-2), and DoubleRow fp8 requires both operands fp8.
  - fc stacks the pair's 128 embedding dims as a single K=128 contraction.
    Matmul out free dim is capped at one PSUM bank (512 f32) by the ISA,
    so no wider instruction merging is possible anywhere.

Per-pair pipeline:
  e_h[k, q]   = (S_Q*KT_h).T @ Q8T_h  row-tiled pair, PSUM [128, 512]
  a_h = exp(e_h * 32/32)              ScalarE / custom-DVE, bf16 out
  o_h/den     = [V_h | 1x8].T @ a_h   accumulate 16 key chunks
  X^T_h = o_h * (1/den)               single 8->64 DMA bcast + DVE recip/
                                      mul, drained one op per group (no
                                      bursts; npool ring 6 so 3 norm
                                      instances overlap their drains)
  y += Xpair^T.T @ Wpair^T            K=128 per pair, 2 pairs in PSUM;
                                      yp stores alternate gpsimd/sync DMA
                                      queues (the 4MB/rep store was
                                      serializing on one queue ~12.6us,
                                      co-binding the fc phase)

Scheduling (hw-swept): attV defers TWO groups behind its exp (absorbs exp
jitter before the PE needs the a-tile); v tiles are SBUF-resident across
reps (no per-rep DMA); e-PSUM ring stays at 3x[128,1024] — dropping to 2
costs +55us (the energy->exp->e-ring recycle is the pacing loop of the
attention phase). Norm osb copies alternate ScalarE/DVE (norm_copy=alt)
and the y staging ring is 6 deep — both A/B-swept wins at the final
balance. DVE has no divide ALU op, so the norm stays recip+mul. The DVE
slot pattern is placement-critical: rotating it by one costs +30us.
"""

import time

import numpy as np

N_CORES = 8
NB = 2          # batch
S = 2048        # sequence length
E = 1024        # embed size
H = 16          # heads
D = 64          # head dim
HPC = 4         # heads per core
NPAIR = 2       # head pairs per core
SCALE = float(1.0 / np.sqrt(E))  # softmax scale (embed_size**0.5)

KC = S // 128   # 16 contraction chunks of 128 keys
QB = S // 512   # 4 query blocks of 512

# exp(x) ~= p(t)^8, p monic cubic, t = x pre-scaled by S_Q on the host.
# ScalarE chunks use exp(t * ACT_SCALE) (exact); VectorE chunks use the
# 8-stage custom DVE polynomial (rel err ~2.6e-4).
ALPHA = 6.0 ** (1.0 / 3.0)
S_Q = float(SCALE / (8.0 * ALPHA))   # host pre-scale on Q (t = S_Q/SCALE * x)
ACT_SCALE = float(8.0 * ALPHA)
B2, B1, B0 = 1.6574587989430332, 1.8171403999384372, 0.9999891634709047


def register_exp_op():
    """Register the EXP_POLY8_ANT custom DVE op (idempotent)."""
    import concourse.dve_ops as dve_ops
    from concourse.dve_ops import OPS, DveOp
    from concourse.dve_spec import C0, C1, C2, Spec, Src0, _has_src1, lower, sq
    from concourse.dve_uop import DveOpSpec

    name = "EXP_POLY8_ANT"
    for op in OPS:
        if op.name == name:
            return op

    _p = ((Src0 + C0) * Src0 + C1) * Src0 + C2
    _body = sq(sq(sq(_p)))

    def _ref(in0, in1, s0, s1, imm2):
        p = ((in0 + s0) * in0 + s1) * in0 + imm2
        return ((p ** 2) ** 2) ** 2

    spec = Spec(body=_body, reference=_ref)
    opcode = dve_ops._CUSTOM_DVE_ROW_BASE + len(OPS)
    shas = {}
    for ver in ("v3", "v4"):
        d = DveOpSpec(
            name=name, opcode=opcode, uops=lower(spec, ver=ver),
            rd1_en=_has_src1(spec),
        )
        shas[ver] = d.sha(ver)
    op = DveOp(name, spec, subdim=False, uops_sha=shas)
    OPS.append(op)
    dve_ops._SUB_OPCODE_FOR_NAME[name] = opcode
    dve_ops.CUSTOM_DVE_SPECS[name] = spec
    return op


DVE_SLOTS = frozenset({0, 2, 4, 5, 7, 9, 11, 12, 14})  # 9/16 of exps on DVE: ScalarE became the tighter engine once fp8-q halved the energy matmuls (A/B-swept on hw)


def build_kernel(reps=1, dve_slots=DVE_SLOTS, abufs=10, ybufs=6, abl=frozenset(),
                 norm_copy="alt", ebufs=3, att_defer=2, norm_div=False,
                 v_resident=True, nbufs=6, yq="half"):
    """abl: timing-ablation flags (numerics become garbage, graph stays valid):
    'attv' = only 2/16 attV chunks; 'exp' = skip exps (attV reads static a);
    'energy' = skip energy matmuls (exps read static sbuf); 'fc' = 2/4 fc
    matmuls; 'norm' = skip norm (fc reads pre-zeroed xt)."""
    import contextlib

    import concourse.bacc as bacc
    import concourse.bass as bass
    import concourse.tile as tile
    from concourse import mybir
    from concourse.alu_op_type import AluOpType

    F32 = mybir.dt.float32
    BF16 = mybir.dt.bfloat16
    FP8 = mybir.dt.float8e4

    exp_op = register_exp_op()

    nc = bacc.Bacc("TRN2", target_bir_lowering=False, num_devices=N_CORES)

    # pair-stacked layouts: rows 0-63 = head 2p, rows 64-127 = head 2p+1
    # q is fp8 (moving operand streams 2 elem/cycle -> energy matmuls 2x);
    # the S_Q prescale lives on k (bf16, exact same rounding count as before).
    qt = nc.dram_tensor("qt", [NPAIR, 128, S], FP8, kind="ExternalInput")
    kt = nc.dram_tensor("kt", [NPAIR, 128, S], BF16, kind="ExternalInput")
    # v carries EIGHT ones columns: the attV matmul then writes den to psum
    # rows 64-71 (free: matmul cost is free-dim-bound, extra out partitions
    # are free), eliminating the 1->8 broadcast DMA hop from the norm chain
    vb = nc.dram_tensor("vb", [HPC, S, D + 8], BF16, kind="ExternalInput")
    wt = nc.dram_tensor("wt", [NPAIR, 128, E], BF16, kind="ExternalInput")
    tok = nc.dram_tensor("tok", [1, 128], F32, kind="ExternalInput")
    yp = nc.dram_tensor("yp", [S, E], BF16, kind="ExternalOutput")
    tok_out = nc.dram_tensor("tok_out", [1, 128], F32, kind="ExternalOutput")

    with tile.TileContext(nc) as tc:
        with contextlib.ExitStack() as ctx:
            singles = ctx.enter_context(tc.tile_pool(name="singles", bufs=1))
            vpool = ctx.enter_context(tc.tile_pool(name="vpool", bufs=2))
            epool = ctx.enter_context(
                tc.tile_pool(name="epool", bufs=ebufs, space="PSUM")
            )
            opool = ctx.enter_context(
                tc.tile_pool(name="opool", bufs=2, space="PSUM")
            )
            apool = ctx.enter_context(tc.tile_pool(name="apool", bufs=abufs))
            npool = ctx.enter_context(tc.tile_pool(name="npool", bufs=nbufs))
            ysb_pool = ctx.enter_context(tc.tile_pool(name="ysb", bufs=ybufs))

            # token passthrough for timing chains
            tok_sb = singles.tile([1, 128], F32)
            nc.gpsimd.dma_start(out=tok_sb, in_=tok[:, :])
            nc.gpsimd.dma_start(out=tok_out[:, :], in_=tok_sb)
            # dummy exp: hoists the ACT_TABLE_LOAD out of the reps loop
            warm_exp = singles.tile([1, 128], BF16)
            nc.scalar.activation(
                out=warm_exp, in_=tok_sb,
                func=mybir.ActivationFunctionType.Exp, scale=1.0,
            )

            # resident inputs (pair-stacked on partitions)
            qt_sb, kt_sb, wt_sb, xt_sb = [], [], [], []
            for p in range(NPAIR):
                q_t = singles.tile([128, S], FP8, tag=f"qt{p}")
                nc.sync.dma_start(out=q_t, in_=qt[p])
                qt_sb.append(q_t)
                k_t = singles.tile([128, S], BF16, tag=f"kt{p}")
                nc.sync.dma_start(out=k_t, in_=kt[p])
                kt_sb.append(k_t)
                w_t = singles.tile([128, E], BF16, tag=f"wt{p}")
                nc.sync.dma_start(out=w_t, in_=wt[p])
                wt_sb.append(w_t)
                # per-(pair, qb) x tiles: fc readers then only wait on the
                # norm that wrote their own query block, not all 8 norms
                xq = []
                for qb in range(QB):
                    x_t = singles.tile([128, 512], BF16, tag=f"xt{p}_{qb}")
                    if "norm" in abl:
                        nc.vector.memset(x_t, 0.0)
                    xq.append(x_t)
                xt_sb.append(xq)

            vres = []
            if v_resident:
                for h in range(HPC):
                    v_r = singles.tile([128, KC, D + 8], BF16, tag=f"vres{h}")
                    nc.sync.dma_start(
                        out=v_r,
                        in_=vb[h].rearrange("(kc q) c -> q kc c", q=128),
                    )
                    vres.append(v_r)

            if "exp" in abl:
                abl_a0 = singles.tile([128, 1024], BF16, tag="abl_a0")
                nc.vector.memset(abl_a0, 0.25)
                abl_a1 = singles.tile([128, 1024], BF16, tag="abl_a1")
                nc.vector.memset(abl_a1, 0.25)
            if "energy" in abl:
                abl_e = singles.tile([128, 1024], F32, tag="abl_e")
                nc.vector.memset(abl_e, 0.125)

            unroll = 4 if reps > 1 and reps % 4 == 0 else (2 if reps > 1 and reps % 2 == 0 else 1)
            loop_cm = (
                tc.For_i(0, reps // unroll, 1)
                if reps > unroll
                else contextlib.nullcontext()
            )
            ctx.enter_context(loop_cm)

            from collections import deque

            for _u in range(unroll):
                exp_idx = 0
                att_q = deque()   # deferred attV groups (att_defer behind)
                norm_work = deque()  # one norm op drained per group (burst smoothing)

                def make_att(g, v0_t, v1_t, a0_t, a1_t, o0_t, o1_t):
                    # head-packed a tiles: a0_t/a1_t hold kc=2g / kc=2g+1,
                    # each with head0 in cols 0-511 and head1 in 512-1023
                    def emit():
                        for j, a_t in ((0, a0_t), (1, a1_t)):
                            kc = 2 * g + j
                            if "attv" in abl and kc not in (0, KC - 1):
                                continue
                            nc.tensor.matmul(
                                o0_t,
                                lhsT=(v0_t[:, kc, :]),
                                rhs=(a_t[:, 0:512]),
                                start=(kc == 0), stop=(kc == KC - 1),
                            )
                            nc.tensor.matmul(
                                o1_t,
                                lhsT=(v1_t[:, kc, :]),
                                rhs=(a_t[:, 512:1024]),
                                start=(kc == 0), stop=(kc == KC - 1),
                            )
                    return emit

                def make_norm_thunks(p, qb, o0_t, o1_t):
                    # 6 thunks drained one per group: copy+bcast per o tile
                    # (freeing the o ring fast), then recips, then muls.
                    # Smoothing avoids the 2.4us DVE burst that stalled the
                    # exp pipeline (and with it attV / the PE stream).
                    otiles = (o0_t, o1_t)
                    osbs = [None, None]
                    bcasts = [None, None]
                    recs = [None, None]

                    def cp(i):
                        def t():
                            osb = npool.tile([D + 8, 512], F32, tag="osb")
                            if norm_copy == "scalar" or (
                                norm_copy == "alt" and i == 0
                            ):
                                nc.scalar.copy(out=osb, in_=otiles[i])
                            else:
                                nc.vector.tensor_copy(osb, otiles[i])
                            osbs[i] = osb
                            # den already sits on 8 partitions (rows 64-71,
                            # written by the 8 ones columns of v): a single
                            # 8 -> 64 DMA with 8 parallel source partitions
                            # finishes the broadcast.
                            bc8 = osb[D : D + 8, :]
                            bcast = npool.tile([D, 512], F32, tag="bcast")
                            bc8_rep = bass.AP(
                                tensor=bc8.tensor,
                                offset=bc8.offset,
                                ap=[list(bc8.ap[0]), [0, 8]]
                                + [list(x) for x in bc8.ap[1:]],
                            )
                            nc.sync.dma_start(out=bcast, in_=bc8_rep)
                            bcasts[i] = bcast
                        return t

                    def rc(i):
                        def t():
                            # Deprioritized: the recip waits on the broadcast
                            # DMAs; exps queued behind it in the DVE FIFO
                            # would head-of-line block and stall attV.
                            with tc.high_priority(offset=-64):
                                rec = npool.tile([D, 512], F32, tag="rec")
                                nc.vector.reciprocal_approx_fast(
                                    out=rec, in_=bcasts[i]
                                )
                                recs[i] = rec
                        return t

                    def ml(i):
                        def t():
                            with tc.high_priority(offset=-64):
                                nc.vector.tensor_mul(
                                    out=xt_sb[p][qb][i * D : (i + 1) * D, :],
                                    in0=osbs[i][0:D, :],
                                    in1=recs[i],
                                )
                        return t

                    def dv(i):
                        def t():
                            # fused o/den in one DVE tensor_tensor divide
                            with tc.high_priority(offset=-64):
                                nc.vector.tensor_tensor(
                                    out=xt_sb[p][qb][i * D : (i + 1) * D, :],
                                    in0=osbs[i][0:D, :],
                                    in1=bcasts[i],
                                    op=AluOpType.divide,
                                )
                        return t

                    if norm_div:
                        return [cp(0), cp(1), dv(0), dv(1)]
                    return [cp(0), cp(1), rc(0), rc(1), ml(0), ml(1)]

                for p in range(NPAIR):
                    if v_resident:
                        # all 4 heads' v tiles fit SBUF (8.3KB/partition):
                        # load once, no per-rep DMA or WAR coupling
                        v0_t, v1_t = vres[2 * p], vres[2 * p + 1]
                    else:
                        # V tiles for this pair's heads: [128, kc, 65] each
                        v0_t = vpool.tile([128, KC, D + 8], BF16, tag="v0")
                        nc.sync.dma_start(
                            out=v0_t,
                            in_=vb[2 * p].rearrange("(kc q) c -> q kc c", q=128),
                        )
                        v1_t = vpool.tile([128, KC, D + 8], BF16, tag="v1")
                        nc.sync.dma_start(
                            out=v1_t,
                            in_=vb[2 * p + 1].rearrange("(kc q) c -> q kc c", q=128),
                        )
                    for qb in range(QB):
                        o0_t = opool.tile([D + 8, 512], F32, tag="o", name="o0")
                        o1_t = opool.tile([D + 8, 512], F32, tag="o", name="o1")
                        # interleaved: per group g (2 key chunks), emit the
                        # row-tiled energy pair matmuls + exps for g, then the
                        # attV matmuls for g-1 (one group behind, so the exps
                        # have a group of slack). Keeps all 3 engines fed.
                        for g in range(KC // 2):
                            if "energy" not in abl:
                                # head-packed e tiles: e tile = ONE kc chunk,
                                # head0 in cols 0-511 / head1 in 512-1023,
                                # written by the CONCURRENT row-tiled pair —
                                # each exp's input completes after one pair-op
                                # instead of two sequential ones
                                e0_t = epool.tile([128, 1024], F32, tag="e", name="e0")
                                e1_t = epool.tile([128, 1024], F32, tag="e", name="e1")
                                for j, e_t in ((0, e0_t), (1, e1_t)):
                                    kc = 2 * g + j
                                    nc.tensor.matmul(
                                        e_t[:, 0:512],
                                        lhsT=(kt_sb[p][0:D, kc * 128 : (kc + 1) * 128]),
                                        rhs=(qt_sb[p][0:D, qb * 512 : (qb + 1) * 512]),
                                        start=True, stop=True,
                                        tile_position=(0, 0),
                                    )
                                    nc.tensor.matmul(
                                        e_t[:, 512:1024],
                                        lhsT=(
                                            kt_sb[p][D : 2 * D, kc * 128 : (kc + 1) * 128]
                                        ),
                                        rhs=(qt_sb[p][D : 2 * D, qb * 512 : (qb + 1) * 512]),
                                        start=True, stop=True,
                                        tile_position=(64, 0),
                                    )
                            else:
                                e0_t = e1_t = abl_e
                            if "exp" not in abl:
                                a0_t = apool.tile([128, 1024], BF16, tag="a0")
                                a1_t = apool.tile([128, 1024], BF16, tag="a1")
                                for e_t, a_t in ((e0_t, a0_t), (e1_t, a1_t)):
                                    if (exp_idx % 16) not in dve_slots:
                                        nc.scalar.activation(
                                            out=a_t,
                                            in_=e_t,
                                            func=mybir.ActivationFunctionType.Exp,
                                            scale=ACT_SCALE,
                                        )
                                    else:
                                        nc.vector._custom_dve(
                                            exp_op, out=a_t, in0=e_t,
                                            s0=B2, s1=B1, imm2=B0,
                                        )
                                    exp_idx += 1
                            else:
                                a0_t, a1_t = abl_a0, abl_a1
                            if len(att_q) >= att_defer:
                                att_q.popleft()()
                            att_q.append(make_att(g, v0_t, v1_t, a0_t, a1_t,
                                                  o0_t, o1_t))
                            # drain norm work AFTER the deferred attV flush
                            # (copies must follow the block's final attV
                            # chunks, which flush at g = att_defer-1). Both
                            # copies go at the first drain group — before the
                            # next attV reuses the o ring — then the DVE ops
                            # smooth out one per group to avoid bursts.
                            if norm_work and g >= att_defer - 1:
                                norm_work.popleft()()
                                if g == att_defer - 1 and norm_work:
                                    norm_work.popleft()()
                        if "norm" not in abl:
                            norm_work.extend(make_norm_thunks(p, qb, o0_t, o1_t))
                # flush the tail: last attV groups, then the remaining norms
                while att_q:
                    att_q.popleft()()
                while norm_work:
                    norm_work.popleft()()

                # fc phase: y[q, f] partial over this core's 2 head-pairs.
                # y PSUM tiles reuse the energy tag (epool) to stay in 8 banks;
                # bf16 stores go out on the (otherwise idle) gpsimd DMA queue so
                # they don't serialize ahead of the next rep's V loads.
                fc_pairs = 1 if "fc" in abl else NPAIR
                for q128 in range(S // 128):
                    y_t = epool.tile([128, 1024], F32, tag="e", name="y_t")
                    for f in range(E // 512):
                        for p in range(fc_pairs):
                            nc.tensor.matmul(
                                y_t[:, f * 512 : (f + 1) * 512],
                                lhsT=(
                                    xt_sb[p][q128 // 4][
                                        :, (q128 % 4) * 128 : (q128 % 4 + 1) * 128
                                    ]
                                ),
                                rhs=(wt_sb[p][:, f * 512 : (f + 1) * 512]),
                                start=(p == 0),
                                stop=(p == fc_pairs - 1),
                            )
                    y_sb = ysb_pool.tile([128, 1024], BF16)
                    if q128 % 2 == 0:
                        nc.scalar.copy(out=y_sb, in_=y_t)
                    else:
                        nc.vector.tensor_copy(y_sb, y_t)
                    # split output DMA across two queues: sync takes the
                    # FIRST half (drains early, so it is empty again when the
                    # next rep's norm bcasts arrive), gpsimd takes the second
                    # half (nothing else uses it at the rep boundary)
                    if yq == "half":
                        dma_q = nc.sync if q128 < 8 else nc.gpsimd
                    else:
                        dma_q = nc.gpsimd if q128 % 2 == 0 else nc.sync
                    dma_q.dma_start(
                        out=yp[q128 * 128 : (q128 + 1) * 128, :],
                        in_=y_sb,
                    )
    nc.compile()
    return nc


class SpmdRunner:
    """Build one jitted shard_map callable over 8 cores; reusable for timing."""

    def __init__(self, nc, n_cores):
        import jax
        from jax.experimental.shard_map import shard_map
        from jax.sharding import Mesh, PartitionSpec

        from concourse import mybir
        from concourse.bass2jax import _bass_exec_p, install_neuronx_cc_hook
        from concourse.bass2jax import partition_id_tensor as _pid

        install_neuronx_cc_hook()
        self.jax = jax
        self.nc = nc
        self.n_cores = n_cores
        self.PartitionSpec = PartitionSpec

        partition_name = nc.partition_id_tensor.name if nc.partition_id_tensor else None
        in_names, out_names, out_avals = [], [], []
        for alloc in nc.m.functions[0].allocations:
            if not isinstance(alloc, mybir.MemoryLocationSet):
                continue
            name = alloc.memorylocations[0].name
            if alloc.kind == "ExternalInput":
                if name != partition_name:
                    in_names.append(name)
            elif alloc.kind == "ExternalOutput":
                out_names.append(name)
                shape = tuple(alloc.tensor_shape)
                dtype = mybir.dt.np(alloc.dtype)
                out_avals.append(jax.core.ShapedArray(shape, dtype))
        self.in_names = in_names
        self.out_names = out_names
        self.out_avals = out_avals
        n_params = len(in_names)
        n_outs = len(out_avals)

        all_in_names = list(in_names) + list(out_names)
        if partition_name is not None:
            all_in_names.append(partition_name)

        def _body(*args):
            operands = list(args)
            if partition_name is not None:
                operands.append(_pid())
            outs = _bass_exec_p.bind(
                *operands,
                out_avals=tuple(out_avals),
                in_names=tuple(all_in_names),
                out_names=tuple(out_names),
                lowering_input_output_aliases=(),
                sim_require_finite=True,
                sim_require_nnan=True,
                nc=nc,
            )
            return tuple(outs)

        self._body = _body
        devices = jax.devices()[:n_cores]
        assert len(devices) == n_cores
        self.mesh = Mesh(np.asarray(devices), ("core",))
        in_specs = (PartitionSpec("core"),) * (n_params + n_outs)
        out_specs = (PartitionSpec("core"),) * n_outs
        self.fn = jax.jit(
            shard_map(
                _body,
                mesh=self.mesh,
                in_specs=in_specs,
                out_specs=out_specs,
                check_rep=False,
            ),
            keep_unused=True,
        )
        self._chain_fns = {}

    def prepare(self, in_maps):
        jax = self.jax
        n = self.n_cores
        concat_in = [
            np.concatenate([np.asarray(in_maps[c][name]) for c in range(n)], axis=0)
            for name in self.in_names
        ]
        concat_zeros = [
            np.zeros((n * a.shape[0], *a.shape[1:]), a.dtype) for a in self.out_avals
        ]
        sharding = jax.sharding.NamedSharding(self.mesh, self.PartitionSpec("core"))
        self.dev_args = [jax.device_put(a, sharding) for a in concat_in + concat_zeros]
        return self.dev_args

    def run(self):
        outs = self.fn(*self.dev_args)
        self.jax.block_until_ready(outs)
        return outs

    def results(self, outs):
        n = self.n_cores
        res = []
        for c in range(n):
            d = {}
            for i, name in enumerate(self.out_names):
                a = np.asarray(outs[i])
                d[name] = a.reshape(n, *self.out_avals[i].shape)[c]
            res.append(d)
        return res

    # ---- timing support: chain K invocations through the tok tensor ----
    def chain_fn(self, k):
        if k in self._chain_fns:
            return self._chain_fns[k]
        jax = self.jax
        from jax.experimental.shard_map import shard_map

        tok_in_idx = self.in_names.index("tok")
        tok_out_idx = self.out_names.index("tok_out")
        n_params = len(self.in_names)

        def _chained(*args):
            args = list(args)
            outs = None
            for _ in range(k):
                outs = self._body(*args)
                args[tok_in_idx] = outs[tok_out_idx]
            return tuple(outs)

        in_specs = (self.PartitionSpec("core"),) * (n_params + len(self.out_names))
        out_specs = (self.PartitionSpec("core"),) * len(self.out_names)
        fn = jax.jit(
            shard_map(
                _chained,
                mesh=self.mesh,
                in_specs=in_specs,
                out_specs=out_specs,
                check_rep=False,
            ),
            keep_unused=True,
        )
        self._chain_fns[k] = fn
        return fn

    def time_chain(self, k, iters=8, warmup=2):
        fn = self.chain_fn(k)
        for _ in range(warmup):
            self.jax.block_until_ready(fn(*self.dev_args))
        ts = []
        for _ in range(iters):
            t0 = time.perf_counter()
            self.jax.block_until_ready(fn(*self.dev_args))
            ts.append(time.perf_counter() - t0)
        return min(ts)


def shard_inputs(values, keys, query, W_out):
    """Build the 8 per-core input maps (host-side layout prep)."""
    import ml_dtypes

    BF = ml_dtypes.bfloat16
    F8 = ml_dtypes.float8_e4m3
    v4 = np.asarray(values, np.float32).reshape(NB, S, H, D)
    k4 = np.asarray(keys, np.float32).reshape(NB, S, H, D)
    q4 = np.asarray(query, np.float32).reshape(NB, S, H, D)
    W_out = np.asarray(W_out, np.float32)
    in_maps = []
    tok = np.zeros((1, 128), np.float32)
    for c in range(N_CORES):
        n = c // 4
        h0 = HPC * (c % 4)
        # [HPC, D, S] per-head transposed views. q ships unscaled in fp8
        # (moving operand); k carries the S_Q prescale in bf16 so energies
        # still arrive as t for the exp paths.
        qh = np.ascontiguousarray(q4[n, :, h0 : h0 + HPC, :].transpose(1, 2, 0))
        kh = np.ascontiguousarray(
            k4[n, :, h0 : h0 + HPC, :].transpose(1, 2, 0) * np.float32(S_Q)
        )
        # pair-stack: [NPAIR, 2*D, S]
        qt = qh.reshape(NPAIR, 2 * D, S).astype(F8)
        kt = kh.reshape(NPAIR, 2 * D, S).astype(BF)
        vb = np.concatenate(
            [
                np.ascontiguousarray(v4[n, :, h0 : h0 + HPC, :].transpose(1, 0, 2)),
                np.ones((HPC, S, 8), np.float32),
            ],
            axis=2,
        ).astype(BF)  # [HPC, S, D+8]: 8 ones cols -> den lands on 8 psum rows
        wt = np.ascontiguousarray(
            W_out[:, (h0 * D) : (h0 + HPC) * D].T.reshape(NPAIR, 2 * D, E)
        ).astype(BF)
        in_maps.append({"qt": qt, "kt": kt, "vb": vb, "wt": wt, "tok": tok})
    return in_maps


_CACHE = {}


def get_runner():
    if "runner" not in _CACHE:
        nc = build_kernel()
        _CACHE["runner"] = SpmdRunner(nc, N_CORES)
    return _CACHE["runner"]


def kernel(values, keys, query, W_out, b_out):
    runner = get_runner()
    in_maps = shard_inputs(values, keys, query, W_out)
    runner.prepare(in_maps)
    outs = runner.run()
    res = runner.results(outs)
    y = np.zeros((NB, S, E), np.float32)
    for c in range(N_CORES):
        y[c // 4] += np.asarray(res[c]["yp"], dtype=np.float32)
    y += np.asarray(b_out, np.float32)[None, None, :]
    return y

